# revision 27
# baseline (speedup 1.0000x reference)
"""GATv2 layer (broadcast-score variant) as a Bass/Tile kernel on 8 NeuronCores.

Math: since scores[i,j] = e[j] (row-broadcast) masked by A, the masked softmax +
aggregation collapse to
    g = exp(e - ln2),  e = relu(X @ W.T) @ a_w          (the ln2 bias cancels)
    out = relu( (A @ (g*Wh)) / (A @ g) )                with Wh = X @ W.T
Each core computes a 1024-row block of the output.

v2: A and G = [g*Wh | g] are fp8e4 (A is a 0/1 mask -> exact); phase 2 uses
DoubleRow perf mode (2 j-chunks per matmul pass, 0.5 cyc/row) which cuts both
the dominant PE stream time and the A DMA traffic in half vs the bf16 version.
Phase 1 runs in bf16 with the g-scaling fused into the PSUM->SBUF copy
(activation Copy with per-partition scale), split across Scalar and DVE.
"""

import os

import numpy as np

import concourse.tile as tile
from concourse import bacc, mybir
from concourse.bass_utils import run_bass_kernel_spmd

N, IN_DIM, OUT_DIM = 8192, 256, 128
NCORES = 8
RPC = N // NCORES          # rows per core (1024)
P = 128                    # partitions
NJ = N // P                # 64 contraction chunks
DH = IN_DIM // P           # 2 chunks of the d-contraction
HF = RPC // 2              # 512-wide i-halves for phase-2 streams
GW = OUT_DIM + 1           # 129 G columns (gWh | g)
GP = 132                   # G pitch (4B aligned)
XS = 8                     # xt DMA slices
CPS = NJ // XS             # chunks per xt slice (8)
LN2 = 0.6931471805599453

F32 = mybir.dt.float32
BF16 = mybir.dt.bfloat16
FP8 = mybir.dt.float8e4
AFT = mybir.ActivationFunctionType
XDT = {"fp8": FP8, "bf16": BF16}[os.environ.get("XDT", "fp8")]

B1 = int(os.environ.get("B1", "4"))    # phase-1 j-tile batch
NB = NJ // B1
ATBUFS = int(os.environ.get("ATBUFS", "32"))
DR = os.environ.get("DR", "1") == "1"  # DoubleRow fp8 phase 2
GENG = os.environ.get("GENG", "alt")   # G-copy engine: alt|act|dve


def emit_body(nc, tc, io, pools):
    at, xt, wt, awb, out = io
    big, atp, ph1, outp = pools

    # Gg memset first: it runs on the (slow-to-launch) GpSimd queue and must
    # beat the first dn matmul.
    Gg = big.tile([P, NJ, OUT_DIM], FP8, tag="Gg", name="Gg")
    nc.gpsimd.memset(Gg, 0.0)
    # small weight tensors FIRST: the first Wh matmul needs wt + xt slice 0,
    # and the DMA queue drains in program order. Early at2 pair DMAs are
    # interleaved among the xt slices so phase 2 isn't starved at its start.
    wt_sb = big.tile([P, DH, OUT_DIM], BF16, tag="wt_sb", name="wt_sb")
    nc.sync.dma_start(out=wt_sb, in_=wt.rearrange("(dh p) o -> p dh o", p=P))
    aw_sb = big.tile([P, B1, OUT_DIM], BF16, tag="aw_sb", name="aw_sb")
    nc.sync.dma_start(out=aw_sb, in_=awb.rearrange("p (b o) -> p b o", b=B1))
    xt_sb = big.tile([P, DH, XS, N // XS], XDT, tag="xt_sb", name="xt_sb")
    xt_r = xt.rearrange("(dh p) (s n) -> p dh s n", p=P, s=XS)
    at2s = []

    def at2_dma(cp):
        at2 = atp.tile([P, 2, RPC], FP8, tag="at2", name="at2")
        nc.sync.dma_start(
            out=at2,
            in_=at[cp * 2 * P:(cp + 1) * 2 * P, :].rearrange(
                "(two p) n -> p two n", p=P),
        )
        at2s.append(at2)

    for s in range(XS):
        nc.sync.dma_start(out=xt_sb[:, :, s, :], in_=xt_r[:, :, s, :])
        if s >= 1:
            at2_dma(s - 1)
    # G = g*Wh contiguous; Gg = [g | 0...0] x NJ for dn (DoubleRow LDWEIGHTS
    # only codegens for contiguous full-width M=128 weights, so g gets 127
    # zero-padded cols; dn PSUM rows 1..127 are never read).
    G = big.tile([P, NJ, OUT_DIM], FP8, tag="G", name="G")
    g64 = big.tile([P, NJ], F32, tag="g64", name="g64")
    ones_bf = big.tile([1, P], BF16, tag="ones", name="ones")
    nc.vector.memset(ones_bf, 1.0)
    nln2 = big.tile([P, 1], F32, tag="nln2", name="nln2")
    nc.vector.memset(nln2, -LN2)
    rc = big.tile([1, RPC], F32, tag="rc", name="rc")

    with tc.tile_pool(name="ps", bufs=1, space="PSUM") as ps:
        # ---- phase 1: Wh tiles -> e -> g -> G = [g*Wh | g], batched by B1 ----
        for b in range(NB):
            wh4 = ps.tile([P, B1, OUT_DIM], F32, tag="wh4", name="wh4", bufs=2)
            for k in range(B1):
                t = b * B1 + k
                s, off = t // CPS, (t % CPS) * P
                for dh in range(DH):
                    nc.tensor.matmul(
                        wh4[:, k, :],
                        xt_sb[:, dh, s, off:off + P],
                        wt_sb[:, dh, :],
                        start=(dh == 0),
                        stop=(dh == DH - 1),
                    )
            t0 = b * B1
            # e[t] = sum_o relu(Wh)*a_w in ONE fused DVE op per chunk
            scr = ph1.tile([P, B1, OUT_DIM], BF16, name="scr")
            e4 = ph1.tile([P, B1], F32, name="e4")
            for k in range(B1):
                nc.vector.scalar_tensor_tensor(
                    out=scr[:, k, :], in0=wh4[:, k, :], scalar=0.0,
                    in1=aw_sb[:, k, :],
                    op0=mybir.AluOpType.max, op1=mybir.AluOpType.mult,
                    accum_out=e4[:, k:k + 1],
                )
            nc.scalar.activation(g64[:, t0:t0 + B1], e4, AFT.Exp, bias=nln2[:, 0:1])
            for k in range(B1):
                t = t0 + k
                # 5-of-8 on Scalar, 3-of-8 on DVE (DVE also runs the e-path)
                eng = (0 if t % 8 < 5 else 1) if GENG == "alt" else \
                    {"act": 0, "dve": 1}[GENG]
                if eng == 0:
                    nc.scalar.activation(
                        G[:, t, :], wh4[:, k, :], AFT.Copy,
                        scale=g64[:, t:t + 1],
                    )
                else:
                    nc.vector.tensor_scalar_mul(
                        G[:, t, :], wh4[:, k, :], g64[:, t:t + 1]
                    )
            nc.gpsimd.tensor_copy(
                out=Gg[:, t0:t0 + B1, 0:1], in_=g64[:, t0:t0 + B1]
            )

        # ---- phase 2: nmT[o, i] += G[j2, o].T @ AT[j2, i], fp8 DoubleRow ----
        nm = [ps.tile([P, HF], F32, tag=f"nm{h}", name=f"nm{h}", bufs=1)
              for h in range(2)]
        dn = [ps.tile([P, HF], F32, tag=f"dn{h}", name=f"dn{h}", bufs=1)
              for h in range(2)]
        if DR:
            NP2 = NJ // 2
            for cp in range(NP2):
                if cp < len(at2s):
                    at2 = at2s[cp]
                else:
                    at2_dma(cp)
                    at2 = at2s[cp]
                for h in range(2):
                    nc.tensor.matmul(
                        nm[h][:, :],
                        G[:, 2 * cp:2 * cp + 2, :],
                        at2[:, :, h * HF:(h + 1) * HF],
                        start=(cp == 0),
                        stop=(cp == NP2 - 1),
                        perf_mode=mybir.MatmulPerfMode.DoubleRow,
                    )
                for h in range(2):
                    nc.tensor.matmul(
                        dn[h][:, :],
                        Gg[:, 2 * cp:2 * cp + 2, :],
                        at2[:, :, h * HF:(h + 1) * HF],
                        start=(cp == 0),
                        stop=(cp == NP2 - 1),
                        perf_mode=mybir.MatmulPerfMode.DoubleRow,
                    )
        else:
            for c in range(NJ):
                at1 = atp.tile([P, RPC], FP8, tag="at2", name="at2")
                nc.sync.dma_start(out=at1, in_=at[c * P:(c + 1) * P, :])
                for h in range(2):
                    nc.tensor.matmul(
                        nm[h][:, :],
                        G[:, c, 0:OUT_DIM],
                        at1[:, h * HF:(h + 1) * HF],
                        start=(c == 0),
                        stop=(c == NJ - 1),
                    )
                for h in range(2):
                    nc.tensor.matmul(
                        dn[h][0:1, :],
                        G[:, c, OUT_DIM:GW],
                        at1[:, h * HF:(h + 1) * HF],
                        start=(c == 0),
                        stop=(c == NJ - 1),
                    )

        # ---- output: out = relu(nm) * (1/dn) broadcast over o ----
        F32R = mybir.dt.float32r
        for h in range(2):
            nc.vector.reciprocal_approx_fast(
                out=rc[0:1, h * HF:(h + 1) * HF], in_=dn[h][0:1, :]
            )
            rel = outp.tile([P, HF], F32, tag="rel", name="rel")
            nc.scalar.activation(rel, nm[h], AFT.Relu)
            rcb = outp.tile([1, HF], BF16, tag="rcb", name="rcb")
            nc.vector.tensor_copy(out=rcb, in_=rc[0:1, h * HF:(h + 1) * HF])
            rbc = ps.tile([P, HF], F32, tag="rbc", name="rbc", bufs=1)
            nc.tensor.matmul(
                rbc, ones_bf[0:1, 0:P], rcb[0:1, :], start=True, stop=True,
            )
            o_sb = outp.tile([P, HF], BF16, tag="osb", name="osb")
            nc.vector.tensor_mul(o_sb, rel, rbc)
            nc.sync.dma_start(out=out[:, h * HF:(h + 1) * HF], in_=o_sb)


def build_nc(repeat=1):
    nc = bacc.Bacc("TRN2", target_bir_lowering=False)
    at = nc.dram_tensor("at", [N, RPC], FP8, kind="ExternalInput")   # A.T col-block
    xt = nc.dram_tensor("xt", [IN_DIM, N], XDT, kind="ExternalInput")  # X.T
    wt = nc.dram_tensor("wt", [IN_DIM, OUT_DIM], BF16, kind="ExternalInput")  # W.T
    awb = nc.dram_tensor("awb", [P, B1 * OUT_DIM], BF16, kind="ExternalInput")
    out = nc.dram_tensor("out", [OUT_DIM, RPC], BF16, kind="ExternalOutput")  # transposed

    with tile.TileContext(nc) as tc:
        with (
            tc.tile_pool(name="big", bufs=1) as big,
            tc.tile_pool(name="atp", bufs=ATBUFS) as atp,
            tc.tile_pool(name="ph1", bufs=4) as ph1,
            tc.tile_pool(name="outp", bufs=2) as outp,
        ):
            for _ in range(repeat):
                emit_body(nc, tc, (at, xt, wt, awb, out), (big, atp, ph1, outp))
    nc.compile()
    return nc


_NC_CACHE = None


def _get_nc():
    global _NC_CACHE
    if _NC_CACHE is None:
        _NC_CACHE = build_nc()
    return _NC_CACHE


def make_in_maps(X, A, W, a_w):
    X = np.ascontiguousarray(np.asarray(X, dtype=np.float32))
    A = np.ascontiguousarray(np.asarray(A, dtype=np.float32))
    W = np.ascontiguousarray(np.asarray(W, dtype=np.float32))
    a_w = np.ascontiguousarray(np.asarray(a_w, dtype=np.float32))

    bf = mybir.dt.np(BF16)
    f8 = mybir.dt.np(FP8)
    xt = np.ascontiguousarray(X.T.astype(mybir.dt.np(XDT)))   # [256, 8192]
    wt = np.ascontiguousarray(W.T.astype(bf))            # [256, 128]
    awb = np.ascontiguousarray(
        np.broadcast_to(np.tile(a_w, B1)[None, :], (P, B1 * OUT_DIM)).astype(bf)
    )

    in_maps = []
    for c in range(NCORES):
        atb = np.ascontiguousarray(A[c * RPC:(c + 1) * RPC, :].T.astype(f8))
        in_maps.append({"at": atb, "xt": xt, "wt": wt, "awb": awb})
    return in_maps


def kernel_with_results(X, A, W, a_w, trace=False):
    in_maps = make_in_maps(X, A, W, a_w)
    res = run_bass_kernel_spmd(_get_nc(), in_maps, list(range(NCORES)), trace=trace)
    out = np.concatenate(
        [np.ascontiguousarray(r["out"].T) for r in res.results], axis=0
    )
    return out.astype(np.float32), res


def kernel(X, A, W, a_w):
    out, _ = kernel_with_results(X, A, W, a_w)
    return out
